# revision 24
# baseline (speedup 1.0000x reference)
"""DeepSeek-V3 router kernel for Trainium2 (8 NeuronCores, SPMD).

Computes, for x:[8192,7168] f32, kernel:[7168,256] f32, bias:[256] f32:
    scores = sigmoid(x @ kernel)
    s = scores + bias
    group top-2 sums over 8 groups of 32 -> top-4 groups mask
    top-8 experts of masked s -> idx (int32), weights = normalized gathered
    sigmoid scores * 2.5
Returns (weights:[8192,8] f32, topk_idx:[8192,8] int32).

Sharding: x split along tokens across 8 cores (1024 tokens/core); router
weight + bias replicated.

GEMM precision scheme (per-core [1024,7168]@[7168,256]):
  x = xh (fp16) + xl,  w = wh (fp16) + wl
  logit ~= xh*wh (fp16 matmul, full rate)
         + q8(xh)*q8(wl) + q8(xl)*q8(wh)   (fp8e4 DoubleRow, 2x rate)
The fp8 corrections recover ~fp32 selection accuracy at half the cost of
fp16 correction passes. All terms share one descale (matched quant
scales) and accumulate into one PSUM bank; sigmoid reads PSUM directly.

Schedule: the fill is DMA-bound (weights 7.3 MB + first x tiles), so all
inputs stream on ONE queue in consumption order with large descriptors,
and tiles 0/1 defer their fp8 corrections until after both fp16 passes —
the PE chews fp16 work while correction operands stream. xh8 casts for
tiles 0/1 run on the (idle) DVE; later tiles cast on ACT, whose queue
carries no DMAs and so has no completion-counter false deps.
"""
import sys

sys.path.insert(0, "/opt/trn_rl_repo")

import numpy as np
import ml_dtypes

import concourse.bass as bass
import concourse.mybir as mybir
from concourse import bacc
from concourse.tile import TileContext
from concourse import bass_utils

T, D, E = 8192, 7168, 256
N_CORES = 8
TS = T // N_CORES          # tokens per core (1024)
NT = TS // 128             # token tiles per core (8)
KC = D // 128              # contraction chunks (56)
G, EPG = 8, 32             # expert groups, experts per group
TOPK_G, TOP_K = 4, 8
SCALE = 2.5
F32 = mybir.dt.float32
F16 = mybir.dt.float16
F8 = mybir.dt.float8e4
U32 = mybir.dt.uint32
E4M3 = ml_dtypes.float8_e4m3   # TRN fp8e4 encoding (max 240)

SX, SW = 64.0, 1024.0      # fp16 scales for x and w
SXH8 = 16.0                # xh8 = xh * (SXH8/SX), device cast
SXL8 = 512.0               # xl8 = q8((x - xh/SX) * SXL8), host
SWH8 = 128.0               # wh8 = q8(wh) * (SWH8/SW), host
SWL8 = 4096.0              # wl8 = q8((w - wh/SW) * SWL8), host
# matched so every term shares one descale: SXH8*SWL8==SXL8*SWH8==SX*SW
S_ALL = SX * SW            # 65536

_BUILt = {}


def build_nc(trace_scopes=False):
    nc = bacc.Bacc(None, target_bir_lowering=False)
    xh_d = nc.dram_tensor("xh", [NT, 128, KC, 128], F16, kind="ExternalInput")
    xl8_d = nc.dram_tensor("xl8", [NT, 128, KC, 128], F8, kind="ExternalInput")
    wh_d = nc.dram_tensor("wh", [128, KC, E], F16, kind="ExternalInput")
    wh8_d = nc.dram_tensor("wh8", [128, KC, E], F8, kind="ExternalInput")
    wl8_d = nc.dram_tensor("wl8", [128, KC, E], F8, kind="ExternalInput")
    bb = nc.dram_tensor("bb", [128, E], F32, kind="ExternalInput")
    wout = nc.dram_tensor("wout", [NT, 128, TOP_K], F32, kind="ExternalOutput")
    iout = nc.dram_tensor("iout", [NT, 128, TOP_K], U32, kind="ExternalOutput")

    with TileContext(nc) as tc:
        with (
            tc.tile_pool(name="const", bufs=1) as constp,
            tc.tile_pool(name="xin", bufs=2) as xinp,
            tc.tile_pool(name="ps", bufs=3, space="PSUM") as psp,
            tc.tile_pool(name="work", bufs=2) as workp,
            tc.tile_pool(name="small", bufs=2) as smallp,
        ):
            bb_sb = constp.tile([128, E], F32)
            # HAM warm-up: dummy matmuls while input DMAs stream, so the
            # first real matmuls run at 2.4 GHz instead of 1.2
            warm = constp.tile([128, 128], F16, tag="warm")
            nc.vector.memset(warm, 0.0)
            wacc = psp.tile([128, 128], F32, tag="wacc")
            for wi in range(36):
                nc.tensor.matmul(wacc, warm, warm, start=(wi == 0),
                                 stop=(wi == 35))

            wh_sb = constp.tile([128, KC, E], F16, tag="wh")
            wh8_sb = constp.tile([128, KC, E], F8, tag="wh8")
            wl8_sb = constp.tile([128, KC, E], F8, tag="wl8")

            xh = [None] * NT
            xl8 = [None] * NT
            xh8 = [None] * NT

            def dma_x(t, n_slices=1):
                xh[t] = xinp.tile([128, KC, 128], F16, tag="xh", name="xht")
                for q in range(n_slices):
                    qs = slice(q * (KC // n_slices), (q + 1) * (KC // n_slices))
                    nc.sync.dma_start(xh[t][:, qs, :], xh_d[t, :, qs, :])

            def dma_xl(t):
                xl8[t] = xinp.tile([128, KC, 128], F8, tag="xl8", name="xl8t")
                nc.sync.dma_start(xl8[t], xl8_d[t, :, :, :])

            def cast_x(t, eng):
                xh8[t] = xinp.tile([128, KC, 128], F8, tag="xh8", name="xh8t")
                for q in range(2):
                    qs = slice(q * 28, (q + 1) * 28)
                    if eng == "dve":
                        nc.vector.tensor_scalar(
                            xh8[t][:, qs, :], xh[t][:, qs, :], SXH8 / SX,
                            None, op0=mybir.AluOpType.mult)
                    else:
                        nc.scalar.activation(
                            xh8[t][:, qs, :], xh[t][:, qs, :],
                            mybir.ActivationFunctionType.Copy,
                            scale=SXH8 / SX)

            accs = {}

            def warm_block(n):
                # dummy matmuls between data-dependent fill blocks: keep
                # the PE busy so HAM doesn't demote the p-state to 4/8
                wa = psp.tile([128, 128], F32, tag="wacc", name="wa")
                for wi in range(n):
                    nc.tensor.matmul(wa, warm, warm, start=(wi == 0),
                                     stop=(wi == n - 1))

            def a16_block(t, start=True):
                accs[t] = psp.tile([128, E], F32, tag="acc", name="acct")
                for c in range(KC):
                    nc.tensor.matmul(accs[t], xh[t][:, c, :], wh_sb[:, c, :],
                                     start=(start and c == 0), stop=False)

            def hl_block(t):
                for p in range(KC // 2):
                    pl = slice(2 * p, 2 * p + 2)
                    nc.tensor.matmul(accs[t], xh8[t][:, pl, :],
                                     wl8_sb[:, pl, :], start=False, stop=False,
                                     perf_mode=mybir.MatmulPerfMode.DoubleRow)

            def lh_block(t):
                NP = KC // 2
                for p in range(NP):
                    pl = slice(2 * p, 2 * p + 2)
                    nc.tensor.matmul(accs[t], xl8[t][:, pl, :],
                                     wh8_sb[:, pl, :], start=False,
                                     stop=(p == NP - 1),
                                     perf_mode=mybir.MatmulPerfMode.DoubleRow)

            def post_block(t):
                acc = accs.pop(t)
                # sigmoid on ACT (reads PSUM, writes SBUF)
                scores = workp.tile([128, E], F32, tag="scores")
                nc.scalar.activation(scores, acc,
                                     mybir.ActivationFunctionType.Sigmoid,
                                     scale=1.0 / S_ALL)
                s = workp.tile([128, E], F32, tag="s")
                nc.vector.tensor_add(s, scores, bb_sb)

                s3 = s[:].rearrange("p (g q) -> p g q", q=EPG)
                r1 = smallp.tile([128, G], F32, tag="r1")
                nc.vector.tensor_reduce(r1, s3, axis=mybir.AxisListType.X,
                                        op=mybir.AluOpType.max)
                mr = workp.tile([128, E], F32, tag="mr")
                nc.vector.match_replace(mr, r1, s, -1e30)
                r2 = smallp.tile([128, G], F32, tag="r2")
                nc.vector.tensor_reduce(
                    r2, mr[:].rearrange("p (g q) -> p g q", q=EPG),
                    axis=mybir.AxisListType.X, op=mybir.AluOpType.max)
                gs = smallp.tile([128, G], F32, tag="gs")
                nc.vector.tensor_add(gs, r1, r2)
                gs8 = smallp.tile([128, 8], F32, tag="gs8")
                nc.vector.max(gs8, gs)
                s_sel = workp.tile([128, E], F32, tag="s_sel")
                nc.vector.scalar_tensor_tensor(
                    s_sel[:].rearrange("p (g q) -> p g q", q=EPG),
                    gs[:].to_broadcast((128, G, EPG)),
                    gs8[:, TOPK_G - 1:TOPK_G], s3,
                    op0=mybir.AluOpType.is_ge, op1=mybir.AluOpType.mult)

                v8 = smallp.tile([128, 8], F32, tag="v8")
                nc.vector.max(v8, s_sel)
                i8 = smallp.tile([128, 8], U32, tag="i8")
                nc.vector.max_index(i8, v8, s_sel)
                nc.gpsimd.dma_start(iout[t, :, :], i8)

                mark = workp.tile([128, E], F32, tag="mark")
                nc.vector.match_replace(mark, v8, s_sel, 2e30)
                msc = workp.tile([128, E], F32, tag="msc")
                nc.vector.scalar_tensor_tensor(
                    msc, mark, 1e30, scores,
                    op0=mybir.AluOpType.is_ge, op1=mybir.AluOpType.mult)
                sc8 = smallp.tile([128, 8], F32, tag="sc8")
                nc.vector.max(sc8, msc)
                isc8 = smallp.tile([128, 8], U32, tag="isc8")
                nc.vector.max_index(isc8, sc8, msc)

                isc8_mid = isc8[:].rearrange(
                    "p (a k) -> p a k", a=1).to_broadcast((128, 8, 8))
                sc8_mid = sc8[:].rearrange(
                    "p (a k) -> p a k", a=1).to_broadcast((128, 8, 8))
                terms = smallp.tile([128, 8, 8], F32, tag="terms")
                nc.vector.tensor_tensor(
                    terms, i8[:].to_broadcast((128, 8, 8)), isc8_mid,
                    op=mybir.AluOpType.is_equal)
                ssum = smallp.tile([128, 1], F32, tag="ssum")
                nc.vector.scalar_tensor_tensor(
                    terms, terms, 1.0, sc8_mid,
                    op0=mybir.AluOpType.mult, op1=mybir.AluOpType.mult,
                    accum_out=ssum)
                w8 = smallp.tile([128, 8], F32, tag="w8")
                nc.vector.tensor_reduce(w8, terms, axis=mybir.AxisListType.X,
                                        op=mybir.AluOpType.add)
                rec = smallp.tile([128, 1], F32, tag="rec")
                nc.vector.reciprocal(rec, ssum)
                wn = smallp.tile([128, 8], F32, tag="wn")
                nc.vector.tensor_scalar(wn, w8, rec[:, 0:1], SCALE,
                                        op0=mybir.AluOpType.mult,
                                        op1=mybir.AluOpType.mult)

                if t == NT - 1:
                    nc.sync.dma_start(wout[t, :, :], wn)
                else:
                    nc.gpsimd.dma_start(wout[t, :, :], wn)

            # ---- fill: one queue, consumption order, big descriptors ----
            # wh/xh0 interleaved (a16(t0) paces on these), then correction
            # operands and tile 1/2 in the order the PE will need them.
            xh[0] = xinp.tile([128, KC, 128], F16, tag="xh", name="xht")
            for q in range(4):
                qs = slice(q * 14, (q + 1) * 14)
                nc.sync.dma_start(wh_sb[:, qs, :], wh_d[:, qs, :])
                nc.sync.dma_start(xh[0][:, qs, :], xh_d[0, :, qs, :])
            nc.sync.dma_start(wl8_sb, wl8_d[:, :, :])
            nc.sync.dma_start(wh8_sb, wh8_d[:, :, :])
            dma_x(1, n_slices=2)
            dma_xl(0)
            dma_xl(1)
            nc.sync.dma_start(bb_sb, bb[:, :])
            dma_x(2)
            dma_xl(2)

            cast_x(0, "dve")
            cast_x(1, "dve")

            # PE fill: fp16 passes first, corrections when operands land
            a16_block(0)
            warm_block(16)
            hl_block(0)
            warm_block(16)
            a16_block(1)
            warm_block(16)
            lh_block(0)          # closes acc(t0)
            warm_block(16)
            hl_block(1)
            lh_block(1)          # closes acc(t1)
            cast_x(2, "act")
            post_block(0)
            post_block(1)

            # ---- steady state ----
            for t in range(2, NT):
                if t + 1 < NT:
                    dma_x(t + 1)
                    dma_xl(t + 1)
                    cast_x(t + 1, "act")
                a16_block(t)
                hl_block(t)
                lh_block(t)
                post_block(t)
    nc.compile()
    return nc


def _prep_inputs(x, kernel, bias):
    """Host-side shard + retile. Returns list of 8 in_maps."""
    x = np.asarray(x, dtype=np.float32)
    kernel = np.asarray(kernel, dtype=np.float32)
    bias = np.asarray(bias, dtype=np.float32)
    ks = kernel * np.float32(SW)
    wh = ks.astype(np.float16)
    wl8 = ((ks - wh.astype(np.float32)) * np.float32(SWL8 / SW)).astype(E4M3)
    wh8 = (wh.astype(np.float32) * np.float32(SWH8 / SW)).astype(E4M3)
    wh_t = np.ascontiguousarray(wh.reshape(KC, 128, E).transpose(1, 0, 2))
    wh8_t = np.ascontiguousarray(wh8.reshape(KC, 128, E).transpose(1, 0, 2))
    wl8_t = np.ascontiguousarray(wl8.reshape(KC, 128, E).transpose(1, 0, 2))
    bb = np.ascontiguousarray(np.broadcast_to(bias, (128, E)))
    in_maps = []
    for core in range(N_CORES):
        xs = x[core * TS:(core + 1) * TS] * np.float32(SX)       # [1024, 7168]
        xsh = xs.astype(np.float16)
        xl8 = ((xs - xsh.astype(np.float32))
               * np.float32(SXL8 / SX)).astype(E4M3)
        xh_t = np.ascontiguousarray(
            xsh.reshape(NT, 128, KC, 128).transpose(0, 3, 2, 1))
        xl8_t = np.ascontiguousarray(
            xl8.reshape(NT, 128, KC, 128).transpose(0, 3, 2, 1))
        in_maps.append({"xh": xh_t, "xl8": xl8_t, "wh": wh_t,
                        "wh8": wh8_t, "wl8": wl8_t, "bb": bb})
    return in_maps


def kernel(x, kernel, bias, _trace=False):
    if "nc" not in _BUILt:
        _BUILt["nc"] = build_nc()
    nc = _BUILt["nc"]
    in_maps = _prep_inputs(x, kernel, bias)
    res = bass_utils.run_bass_kernel_spmd(
        nc, in_maps, core_ids=list(range(N_CORES)), trace=_trace)
    weights = np.empty((T, TOP_K), np.float32)
    idx = np.empty((T, TOP_K), np.int32)
    for core in range(N_CORES):
        weights[core * TS:(core + 1) * TS] = res.results[core]["wout"].reshape(TS, TOP_K)
        idx[core * TS:(core + 1) * TS] = (
            res.results[core]["iout"].reshape(TS, TOP_K).astype(np.int32))
    _BUILt["last_result"] = res
    return weights, idx


# revision 26
# speedup vs baseline: 1.0184x; 1.0184x over previous
"""DeepSeek-V3 router kernel for Trainium2 (8 NeuronCores, SPMD).

Computes, for x:[8192,7168] f32, kernel:[7168,256] f32, bias:[256] f32:
    scores = sigmoid(x @ kernel)
    s = scores + bias
    group top-2 sums over 8 groups of 32 -> top-4 groups mask
    top-8 experts of masked s -> idx (int32), weights = normalized gathered
    sigmoid scores * 2.5
Returns (weights:[8192,8] f32, topk_idx:[8192,8] int32).

Sharding: x split along tokens across 8 cores (1024 tokens/core); router
weight + bias replicated.

GEMM precision scheme (per-core [1024,7168]@[7168,256]):
  x = xh (fp16) + xl,  w = wh (fp16) + wl
  logit ~= xh*wh (fp16 matmul, full rate)
         + q8(xh)*q8(wl) + q8(xl)*q8(wh)   (fp8e4 DoubleRow, 2x rate)
The fp8 corrections recover ~fp32 selection accuracy at half the cost of
fp16 correction passes. All terms share one descale (matched quant
scales) and accumulate into one PSUM bank; sigmoid reads PSUM directly.

Schedule: the fill is DMA-bound (weights 7.3 MB + first x tiles), so all
inputs stream on ONE queue in consumption order with large descriptors,
and tiles 0/1 defer their fp8 corrections until after both fp16 passes —
the PE chews fp16 work while correction operands stream. xh8 casts for
tiles 0/1 run on the (idle) DVE; later tiles cast on ACT, whose queue
carries no DMAs and so has no completion-counter false deps.
"""
import sys

sys.path.insert(0, "/opt/trn_rl_repo")

import numpy as np
import ml_dtypes

import concourse.bass as bass
import concourse.mybir as mybir
from concourse import bacc
from concourse.tile import TileContext
from concourse import bass_utils

T, D, E = 8192, 7168, 256
N_CORES = 8
TS = T // N_CORES          # tokens per core (1024)
NT = TS // 128             # token tiles per core (8)
KC = D // 128              # contraction chunks (56)
G, EPG = 8, 32             # expert groups, experts per group
TOPK_G, TOP_K = 4, 8
SCALE = 2.5
F32 = mybir.dt.float32
F16 = mybir.dt.float16
F8 = mybir.dt.float8e4
U32 = mybir.dt.uint32
E4M3 = ml_dtypes.float8_e4m3   # TRN fp8e4 encoding (max 240)

SX, SW = 64.0, 1024.0      # fp16 scales for x and w
SXH8 = 16.0                # xh8 = xh * (SXH8/SX), device cast
SXL8 = 512.0               # xl8 = q8((x - xh/SX) * SXL8), host
SWH8 = 128.0               # wh8 = q8(wh) * (SWH8/SW), host
SWL8 = 4096.0              # wl8 = q8((w - wh/SW) * SWL8), host
# matched so every term shares one descale: SXH8*SWL8==SXL8*SWH8==SX*SW
S_ALL = SX * SW            # 65536

_BUILt = {}


def build_nc(trace_scopes=False):
    nc = bacc.Bacc(None, target_bir_lowering=False)
    xh_d = nc.dram_tensor("xh", [NT, 128, KC, 128], F16, kind="ExternalInput")
    xl8_d = nc.dram_tensor("xl8", [NT, 128, KC, 128], F8, kind="ExternalInput")
    wh_d = nc.dram_tensor("wh", [128, KC, E], F16, kind="ExternalInput")
    wh8_d = nc.dram_tensor("wh8", [128, KC, E], F8, kind="ExternalInput")
    wl8_d = nc.dram_tensor("wl8", [128, KC, E], F8, kind="ExternalInput")
    bb = nc.dram_tensor("bb", [128, E], F32, kind="ExternalInput")
    wout = nc.dram_tensor("wout", [NT, 128, TOP_K], F32, kind="ExternalOutput")
    iout = nc.dram_tensor("iout", [NT, 128, TOP_K], U32, kind="ExternalOutput")

    with TileContext(nc) as tc:
        with (
            tc.tile_pool(name="const", bufs=1) as constp,
            tc.tile_pool(name="xin", bufs=2) as xinp,
            tc.tile_pool(name="ps", bufs=3, space="PSUM") as psp,
            tc.tile_pool(name="work", bufs=2) as workp,
            tc.tile_pool(name="small", bufs=2) as smallp,
        ):
            bb_sb = constp.tile([128, E], F32)
            # HAM warm-up: dummy matmuls while input DMAs stream, so the
            # first real matmuls run at 2.4 GHz instead of 1.2
            warm = constp.tile([128, 128], F16, tag="warm")
            nc.vector.memset(warm, 0.0)
            wacc = psp.tile([128, 128], F32, tag="wacc")
            for wi in range(36):
                nc.tensor.matmul(wacc, warm, warm, start=(wi == 0),
                                 stop=(wi == 35))

            wh_sb = constp.tile([128, KC, E], F16, tag="wh")
            wh8_sb = constp.tile([128, KC, E], F8, tag="wh8")
            wl8_sb = constp.tile([128, KC, E], F8, tag="wl8")

            xh = [None] * NT
            xl8 = [None] * NT
            xh8 = [None] * NT

            def dma_x(t, n_slices=1):
                xh[t] = xinp.tile([128, KC, 128], F16, tag="xh", name="xht")
                for q in range(n_slices):
                    qs = slice(q * (KC // n_slices), (q + 1) * (KC // n_slices))
                    nc.sync.dma_start(xh[t][:, qs, :], xh_d[t, :, qs, :])

            def dma_xl(t):
                xl8[t] = xinp.tile([128, KC, 128], F8, tag="xl8", name="xl8t")
                nc.sync.dma_start(xl8[t], xl8_d[t, :, :, :])

            def cast_x(t, eng):
                xh8[t] = xinp.tile([128, KC, 128], F8, tag="xh8", name="xh8t")
                for q in range(2):
                    qs = slice(q * 28, (q + 1) * 28)
                    if eng == "dve":
                        nc.vector.tensor_scalar(
                            xh8[t][:, qs, :], xh[t][:, qs, :], SXH8 / SX,
                            None, op0=mybir.AluOpType.mult)
                    else:
                        nc.scalar.activation(
                            xh8[t][:, qs, :], xh[t][:, qs, :],
                            mybir.ActivationFunctionType.Copy,
                            scale=SXH8 / SX)

            accs = {}

            def warm_block(n):
                # dummy matmuls between data-dependent fill blocks: keep
                # the PE busy so HAM doesn't demote the p-state to 4/8
                wa = psp.tile([128, 128], F32, tag="wacc", name="wa")
                for wi in range(n):
                    nc.tensor.matmul(wa, warm, warm, start=(wi == 0),
                                     stop=(wi == n - 1))

            def a16_block(t, start=True):
                accs[t] = psp.tile([128, E], F32, tag="acc", name="acct")
                for c in range(KC):
                    nc.tensor.matmul(accs[t], xh[t][:, c, :], wh_sb[:, c, :],
                                     start=(start and c == 0), stop=False)

            def hl_block(t):
                for p in range(KC // 2):
                    pl = slice(2 * p, 2 * p + 2)
                    nc.tensor.matmul(accs[t], xh8[t][:, pl, :],
                                     wl8_sb[:, pl, :], start=False, stop=False,
                                     perf_mode=mybir.MatmulPerfMode.DoubleRow)

            def lh_block(t):
                NP = KC // 2
                for p in range(NP):
                    pl = slice(2 * p, 2 * p + 2)
                    nc.tensor.matmul(accs[t], xl8[t][:, pl, :],
                                     wh8_sb[:, pl, :], start=False,
                                     stop=(p == NP - 1),
                                     perf_mode=mybir.MatmulPerfMode.DoubleRow)

            def post_block(t):
                acc = accs.pop(t)
                # sigmoid on ACT (reads PSUM, writes SBUF)
                scores = workp.tile([128, E], F32, tag="scores")
                nc.scalar.activation(scores, acc,
                                     mybir.ActivationFunctionType.Sigmoid,
                                     scale=1.0 / S_ALL)
                s = workp.tile([128, E], F32, tag="s")
                nc.vector.tensor_add(s, scores, bb_sb)

                s3 = s[:].rearrange("p (g q) -> p g q", q=EPG)
                r1 = smallp.tile([128, G], F32, tag="r1")
                nc.vector.tensor_reduce(r1, s3, axis=mybir.AxisListType.X,
                                        op=mybir.AluOpType.max)
                mr = workp.tile([128, E], F32, tag="mr")
                nc.vector.match_replace(mr, r1, s, -1e30)
                r2 = smallp.tile([128, G], F32, tag="r2")
                nc.vector.tensor_reduce(
                    r2, mr[:].rearrange("p (g q) -> p g q", q=EPG),
                    axis=mybir.AxisListType.X, op=mybir.AluOpType.max)
                gs = smallp.tile([128, G], F32, tag="gs")
                nc.vector.tensor_add(gs, r1, r2)
                gs8 = smallp.tile([128, 8], F32, tag="gs8")
                nc.vector.max(gs8, gs)
                s_sel = workp.tile([128, E], F32, tag="s_sel")
                nc.vector.scalar_tensor_tensor(
                    s_sel[:].rearrange("p (g q) -> p g q", q=EPG),
                    gs[:].to_broadcast((128, G, EPG)),
                    gs8[:, TOPK_G - 1:TOPK_G], s3,
                    op0=mybir.AluOpType.is_ge, op1=mybir.AluOpType.mult)

                v8 = smallp.tile([128, 8], F32, tag="v8")
                nc.vector.max(v8, s_sel)
                i8 = smallp.tile([128, 8], U32, tag="i8")
                nc.vector.max_index(i8, v8, s_sel)
                nc.gpsimd.dma_start(iout[t, :, :], i8)

                mark = workp.tile([128, E], F32, tag="mark")
                nc.vector.match_replace(mark, v8, s_sel, 2e30)
                msc = workp.tile([128, E], F32, tag="msc")
                nc.vector.scalar_tensor_tensor(
                    msc, mark, 1e30, scores,
                    op0=mybir.AluOpType.is_ge, op1=mybir.AluOpType.mult)
                sc8 = smallp.tile([128, 8], F32, tag="sc8")
                nc.vector.max(sc8, msc)
                isc8 = smallp.tile([128, 8], U32, tag="isc8")
                nc.vector.max_index(isc8, sc8, msc)

                isc8_mid = isc8[:].rearrange(
                    "p (a k) -> p a k", a=1).to_broadcast((128, 8, 8))
                sc8_mid = sc8[:].rearrange(
                    "p (a k) -> p a k", a=1).to_broadcast((128, 8, 8))
                terms = smallp.tile([128, 8, 8], F32, tag="terms")
                nc.vector.tensor_tensor(
                    terms, i8[:].to_broadcast((128, 8, 8)), isc8_mid,
                    op=mybir.AluOpType.is_equal)
                ssum = smallp.tile([128, 1], F32, tag="ssum")
                nc.vector.scalar_tensor_tensor(
                    terms, terms, 1.0, sc8_mid,
                    op0=mybir.AluOpType.mult, op1=mybir.AluOpType.mult,
                    accum_out=ssum)
                w8 = smallp.tile([128, 8], F32, tag="w8")
                nc.vector.tensor_reduce(w8, terms, axis=mybir.AxisListType.X,
                                        op=mybir.AluOpType.add)
                rec = smallp.tile([128, 1], F32, tag="rec")
                nc.vector.reciprocal(rec, ssum)
                wn = smallp.tile([128, 8], F32, tag="wn")
                nc.vector.tensor_scalar(wn, w8, rec[:, 0:1], SCALE,
                                        op0=mybir.AluOpType.mult,
                                        op1=mybir.AluOpType.mult)

                if t == NT - 1:
                    nc.sync.dma_start(wout[t, :, :], wn)
                else:
                    nc.gpsimd.dma_start(wout[t, :, :], wn)

            # ---- fill: one queue, consumption order, big descriptors ----
            # wh/xh0 interleaved (a16(t0) paces on these), then correction
            # operands and tile 1/2 in the order the PE will need them.
            xh[0] = xinp.tile([128, KC, 128], F16, tag="xh", name="xht")
            for q in range(4):
                qs = slice(q * 14, (q + 1) * 14)
                nc.sync.dma_start(wh_sb[:, qs, :], wh_d[:, qs, :])
                nc.sync.dma_start(xh[0][:, qs, :], xh_d[0, :, qs, :])
            nc.sync.dma_start(wl8_sb, wl8_d[:, :, :])
            dma_x(1, n_slices=2)
            nc.sync.dma_start(wh8_sb, wh8_d[:, :, :])
            dma_xl(0)
            dma_xl(1)
            nc.sync.dma_start(bb_sb, bb[:, :])
            dma_x(2)
            dma_xl(2)

            cast_x(0, "dve")
            cast_x(1, "dve")

            # PE fill: fp16 passes first, corrections when operands land
            a16_block(0)
            warm_block(16)
            hl_block(0)
            warm_block(16)
            a16_block(1)
            lh_block(0)          # closes acc(t0)
            hl_block(1)
            lh_block(1)          # closes acc(t1)
            cast_x(2, "act")
            post_block(0)
            post_block(1)

            # ---- steady state ----
            for t in range(2, NT):
                if t + 1 < NT:
                    dma_x(t + 1)
                    dma_xl(t + 1)
                    cast_x(t + 1, "act")
                a16_block(t)
                hl_block(t)
                lh_block(t)
                post_block(t)
    nc.compile()
    return nc


def _prep_inputs(x, kernel, bias):
    """Host-side shard + retile. Returns list of 8 in_maps."""
    x = np.asarray(x, dtype=np.float32)
    kernel = np.asarray(kernel, dtype=np.float32)
    bias = np.asarray(bias, dtype=np.float32)
    ks = kernel * np.float32(SW)
    wh = ks.astype(np.float16)
    wl8 = ((ks - wh.astype(np.float32)) * np.float32(SWL8 / SW)).astype(E4M3)
    wh8 = (wh.astype(np.float32) * np.float32(SWH8 / SW)).astype(E4M3)
    wh_t = np.ascontiguousarray(wh.reshape(KC, 128, E).transpose(1, 0, 2))
    wh8_t = np.ascontiguousarray(wh8.reshape(KC, 128, E).transpose(1, 0, 2))
    wl8_t = np.ascontiguousarray(wl8.reshape(KC, 128, E).transpose(1, 0, 2))
    bb = np.ascontiguousarray(np.broadcast_to(bias, (128, E)))
    in_maps = []
    for core in range(N_CORES):
        xs = x[core * TS:(core + 1) * TS] * np.float32(SX)       # [1024, 7168]
        xsh = xs.astype(np.float16)
        xl8 = ((xs - xsh.astype(np.float32))
               * np.float32(SXL8 / SX)).astype(E4M3)
        xh_t = np.ascontiguousarray(
            xsh.reshape(NT, 128, KC, 128).transpose(0, 3, 2, 1))
        xl8_t = np.ascontiguousarray(
            xl8.reshape(NT, 128, KC, 128).transpose(0, 3, 2, 1))
        in_maps.append({"xh": xh_t, "xl8": xl8_t, "wh": wh_t,
                        "wh8": wh8_t, "wl8": wl8_t, "bb": bb})
    return in_maps


def kernel(x, kernel, bias, _trace=False):
    if "nc" not in _BUILt:
        _BUILt["nc"] = build_nc()
    nc = _BUILt["nc"]
    in_maps = _prep_inputs(x, kernel, bias)
    res = bass_utils.run_bass_kernel_spmd(
        nc, in_maps, core_ids=list(range(N_CORES)), trace=_trace)
    weights = np.empty((T, TOP_K), np.float32)
    idx = np.empty((T, TOP_K), np.int32)
    for core in range(N_CORES):
        weights[core * TS:(core + 1) * TS] = res.results[core]["wout"].reshape(TS, TOP_K)
        idx[core * TS:(core + 1) * TS] = (
            res.results[core]["iout"].reshape(TS, TOP_K).astype(np.int32))
    _BUILt["last_result"] = res
    return weights, idx


# revision 27
# speedup vs baseline: 1.0624x; 1.0432x over previous
"""DeepSeek-V3 router kernel for Trainium2 (8 NeuronCores, SPMD).

Computes, for x:[8192,7168] f32, kernel:[7168,256] f32, bias:[256] f32:
    scores = sigmoid(x @ kernel)
    s = scores + bias
    group top-2 sums over 8 groups of 32 -> top-4 groups mask
    top-8 experts of masked s -> idx (int32), weights = normalized gathered
    sigmoid scores * 2.5
Returns (weights:[8192,8] f32, topk_idx:[8192,8] int32).

Sharding: x split along tokens across 8 cores (1024 tokens/core); router
weight + bias replicated.

GEMM precision scheme (per-core [1024,7168]@[7168,256]):
  x = xh (fp16) + xl,  w = wh (fp16) + wl
  logit ~= xh*wh (fp16 matmul, full rate)
         + q8(xh)*q8(wl) + q8(xl)*q8(wh)   (fp8e4 DoubleRow, 2x rate)
The fp8 corrections recover ~fp32 selection accuracy at half the cost of
fp16 correction passes. All terms share one descale (matched quant
scales) and accumulate into one PSUM bank; sigmoid reads PSUM directly.

Schedule: the fill is DMA-bound (weights 7.3 MB + first x tiles), so all
inputs stream on ONE queue in consumption order with large descriptors,
and tiles 0/1 defer their fp8 corrections until after both fp16 passes —
the PE chews fp16 work while correction operands stream. xh8 casts for
tiles 0/1 run on the (idle) DVE; later tiles cast on ACT, whose queue
carries no DMAs and so has no completion-counter false deps.
"""
import sys

sys.path.insert(0, "/opt/trn_rl_repo")

import numpy as np
import ml_dtypes

import concourse.bass as bass
import concourse.mybir as mybir
from concourse import bacc
from concourse.tile import TileContext
from concourse import bass_utils

T, D, E = 8192, 7168, 256
N_CORES = 8
TS = T // N_CORES          # tokens per core (1024)
NT = TS // 128             # token tiles per core (8)
KC = D // 128              # contraction chunks (56)
G, EPG = 8, 32             # expert groups, experts per group
TOPK_G, TOP_K = 4, 8
SCALE = 2.5
F32 = mybir.dt.float32
F16 = mybir.dt.float16
F8 = mybir.dt.float8e4
U32 = mybir.dt.uint32
E4M3 = ml_dtypes.float8_e4m3   # TRN fp8e4 encoding (max 240)

SX, SW = 256.0, 1024.0     # fp16 scales for x and w
SXH8 = 16.0                # xh8 = xh * (SXH8/SX), device cast
SXL8 = 2048.0              # xl8 = q8((x - xh/SX) * SXL8), host
SWH8 = 128.0               # wh8 = q8(wh) * (SWH8/SW), host
SWL8 = 16384.0             # wl8 = q8((w - wh/SW) * SWL8), host
# SX=256 keeps all fp8 operands clear of e4m3's subnormal floor
# matched so every term shares one descale: SXH8*SWL8==SXL8*SWH8==SX*SW
S_ALL = SX * SW            # 65536

_BUILt = {}


def build_nc(trace_scopes=False):
    nc = bacc.Bacc(None, target_bir_lowering=False)
    xh_d = nc.dram_tensor("xh", [NT, 128, KC, 128], F16, kind="ExternalInput")
    xl8_d = nc.dram_tensor("xl8", [NT, 128, KC, 128], F8, kind="ExternalInput")
    wh_d = nc.dram_tensor("wh", [128, KC, E], F16, kind="ExternalInput")
    wh8_d = nc.dram_tensor("wh8", [128, KC, E], F8, kind="ExternalInput")
    wl8_d = nc.dram_tensor("wl8", [128, KC, E], F8, kind="ExternalInput")
    bb = nc.dram_tensor("bb", [128, E], F32, kind="ExternalInput")
    wout = nc.dram_tensor("wout", [NT, 128, TOP_K], F32, kind="ExternalOutput")
    iout = nc.dram_tensor("iout", [NT, 128, TOP_K], U32, kind="ExternalOutput")

    with TileContext(nc) as tc:
        with (
            tc.tile_pool(name="const", bufs=1) as constp,
            tc.tile_pool(name="xin", bufs=2) as xinp,
            tc.tile_pool(name="ps", bufs=3, space="PSUM") as psp,
            tc.tile_pool(name="work", bufs=2) as workp,
            tc.tile_pool(name="small", bufs=2) as smallp,
        ):
            bb_sb = constp.tile([128, E], F32)
            # HAM warm-up: dummy matmuls while input DMAs stream, so the
            # first real matmuls run at 2.4 GHz instead of 1.2
            warm = constp.tile([128, 128], F16, tag="warm")
            nc.vector.memset(warm, 0.0)
            wacc = psp.tile([128, 128], F32, tag="wacc")
            for wi in range(36):
                nc.tensor.matmul(wacc, warm, warm, start=(wi == 0),
                                 stop=(wi == 35))

            wh_sb = constp.tile([128, KC, E], F16, tag="wh")
            wh8_sb = constp.tile([128, KC, E], F8, tag="wh8")
            wl8_sb = constp.tile([128, KC, E], F8, tag="wl8")

            xh = [None] * NT
            xl8 = [None] * NT
            xh8 = [None] * NT

            def dma_x(t, n_slices=1):
                xh[t] = xinp.tile([128, KC, 128], F16, tag="xh", name="xht")
                for q in range(n_slices):
                    qs = slice(q * (KC // n_slices), (q + 1) * (KC // n_slices))
                    nc.sync.dma_start(xh[t][:, qs, :], xh_d[t, :, qs, :])

            def dma_xl(t):
                xl8[t] = xinp.tile([128, KC, 128], F8, tag="xl8", name="xl8t")
                nc.sync.dma_start(xl8[t], xl8_d[t, :, :, :])

            def cast_x(t, eng):
                xh8[t] = xinp.tile([128, KC, 128], F8, tag="xh8", name="xh8t")
                for q in range(2):
                    qs = slice(q * 28, (q + 1) * 28)
                    if eng == "dve":
                        nc.vector.tensor_scalar(
                            xh8[t][:, qs, :], xh[t][:, qs, :], SXH8 / SX,
                            None, op0=mybir.AluOpType.mult)
                    else:
                        nc.scalar.activation(
                            xh8[t][:, qs, :], xh[t][:, qs, :],
                            mybir.ActivationFunctionType.Copy,
                            scale=SXH8 / SX)

            accs = {}

            def warm_block(n):
                # dummy matmuls between data-dependent fill blocks: keep
                # the PE busy so HAM doesn't demote the p-state to 4/8
                wa = psp.tile([128, 128], F32, tag="wacc", name="wa")
                for wi in range(n):
                    nc.tensor.matmul(wa, warm, warm, start=(wi == 0),
                                     stop=(wi == n - 1))

            def a16_block(t, start=True):
                accs[t] = psp.tile([128, E], F32, tag="acc", name="acct")
                for c in range(KC):
                    nc.tensor.matmul(accs[t], xh[t][:, c, :], wh_sb[:, c, :],
                                     start=(start and c == 0), stop=False)

            def hl_block(t):
                for p in range(KC // 2):
                    pl = slice(2 * p, 2 * p + 2)
                    nc.tensor.matmul(accs[t], xh8[t][:, pl, :],
                                     wl8_sb[:, pl, :], start=False, stop=False,
                                     perf_mode=mybir.MatmulPerfMode.DoubleRow)

            def lh_block(t):
                NP = KC // 2
                for p in range(NP):
                    pl = slice(2 * p, 2 * p + 2)
                    nc.tensor.matmul(accs[t], xl8[t][:, pl, :],
                                     wh8_sb[:, pl, :], start=False,
                                     stop=(p == NP - 1),
                                     perf_mode=mybir.MatmulPerfMode.DoubleRow)

            def post_block(t):
                acc = accs.pop(t)
                # sigmoid on ACT (reads PSUM, writes SBUF)
                scores = workp.tile([128, E], F32, tag="scores")
                nc.scalar.activation(scores, acc,
                                     mybir.ActivationFunctionType.Sigmoid,
                                     scale=1.0 / S_ALL)
                s = workp.tile([128, E], F32, tag="s")
                nc.vector.tensor_add(s, scores, bb_sb)

                s3 = s[:].rearrange("p (g q) -> p g q", q=EPG)
                r1 = smallp.tile([128, G], F32, tag="r1")
                nc.vector.tensor_reduce(r1, s3, axis=mybir.AxisListType.X,
                                        op=mybir.AluOpType.max)
                mr = workp.tile([128, E], F32, tag="mr")
                nc.vector.match_replace(mr, r1, s, -1e30)
                r2 = smallp.tile([128, G], F32, tag="r2")
                nc.vector.tensor_reduce(
                    r2, mr[:].rearrange("p (g q) -> p g q", q=EPG),
                    axis=mybir.AxisListType.X, op=mybir.AluOpType.max)
                gs = smallp.tile([128, G], F32, tag="gs")
                nc.vector.tensor_add(gs, r1, r2)
                gs8 = smallp.tile([128, 8], F32, tag="gs8")
                nc.vector.max(gs8, gs)
                s_sel = workp.tile([128, E], F32, tag="s_sel")
                nc.vector.scalar_tensor_tensor(
                    s_sel[:].rearrange("p (g q) -> p g q", q=EPG),
                    gs[:].to_broadcast((128, G, EPG)),
                    gs8[:, TOPK_G - 1:TOPK_G], s3,
                    op0=mybir.AluOpType.is_ge, op1=mybir.AluOpType.mult)

                v8 = smallp.tile([128, 8], F32, tag="v8")
                nc.vector.max(v8, s_sel)
                i8 = smallp.tile([128, 8], U32, tag="i8")
                nc.vector.max_index(i8, v8, s_sel)
                nc.gpsimd.dma_start(iout[t, :, :], i8)

                mark = workp.tile([128, E], F32, tag="mark")
                nc.vector.match_replace(mark, v8, s_sel, 2e30)
                msc = workp.tile([128, E], F32, tag="msc")
                nc.vector.scalar_tensor_tensor(
                    msc, mark, 1e30, scores,
                    op0=mybir.AluOpType.is_ge, op1=mybir.AluOpType.mult)
                sc8 = smallp.tile([128, 8], F32, tag="sc8")
                nc.vector.max(sc8, msc)
                isc8 = smallp.tile([128, 8], U32, tag="isc8")
                nc.vector.max_index(isc8, sc8, msc)

                isc8_mid = isc8[:].rearrange(
                    "p (a k) -> p a k", a=1).to_broadcast((128, 8, 8))
                sc8_mid = sc8[:].rearrange(
                    "p (a k) -> p a k", a=1).to_broadcast((128, 8, 8))
                terms = smallp.tile([128, 8, 8], F32, tag="terms")
                nc.vector.tensor_tensor(
                    terms, i8[:].to_broadcast((128, 8, 8)), isc8_mid,
                    op=mybir.AluOpType.is_equal)
                ssum = smallp.tile([128, 1], F32, tag="ssum")
                nc.vector.scalar_tensor_tensor(
                    terms, terms, 1.0, sc8_mid,
                    op0=mybir.AluOpType.mult, op1=mybir.AluOpType.mult,
                    accum_out=ssum)
                w8 = smallp.tile([128, 8], F32, tag="w8")
                nc.vector.tensor_reduce(w8, terms, axis=mybir.AxisListType.X,
                                        op=mybir.AluOpType.add)
                rec = smallp.tile([128, 1], F32, tag="rec")
                nc.vector.reciprocal(rec, ssum)
                wn = smallp.tile([128, 8], F32, tag="wn")
                nc.vector.tensor_scalar(wn, w8, rec[:, 0:1], SCALE,
                                        op0=mybir.AluOpType.mult,
                                        op1=mybir.AluOpType.mult)

                if t == NT - 1:
                    nc.sync.dma_start(wout[t, :, :], wn)
                else:
                    nc.gpsimd.dma_start(wout[t, :, :], wn)

            # ---- fill: one queue, consumption order, big descriptors ----
            # wh/xh0 interleaved (a16(t0) paces on these), then correction
            # operands and tile 1/2 in the order the PE will need them.
            xh[0] = xinp.tile([128, KC, 128], F16, tag="xh", name="xht")
            for q in range(4):
                qs = slice(q * 14, (q + 1) * 14)
                nc.sync.dma_start(wh_sb[:, qs, :], wh_d[:, qs, :])
                nc.sync.dma_start(xh[0][:, qs, :], xh_d[0, :, qs, :])
            nc.sync.dma_start(wl8_sb, wl8_d[:, :, :])
            dma_x(1, n_slices=2)
            nc.sync.dma_start(wh8_sb, wh8_d[:, :, :])
            dma_xl(0)
            dma_xl(1)
            nc.sync.dma_start(bb_sb, bb[:, :])
            dma_x(2)
            dma_xl(2)

            cast_x(0, "dve")
            cast_x(1, "dve")

            # PE fill: fp16 passes first, corrections when operands land
            a16_block(0)
            warm_block(16)
            hl_block(0)
            warm_block(16)
            a16_block(1)
            lh_block(0)          # closes acc(t0)
            hl_block(1)
            lh_block(1)          # closes acc(t1)
            cast_x(2, "act")
            post_block(0)
            post_block(1)

            # ---- steady state ----
            for t in range(2, NT):
                if t + 1 < NT:
                    dma_x(t + 1)
                    dma_xl(t + 1)
                    cast_x(t + 1, "act")
                a16_block(t)
                hl_block(t)
                lh_block(t)
                post_block(t)
    nc.compile()
    return nc


def _prep_inputs(x, kernel, bias):
    """Host-side shard + retile. Returns list of 8 in_maps."""
    x = np.asarray(x, dtype=np.float32)
    kernel = np.asarray(kernel, dtype=np.float32)
    bias = np.asarray(bias, dtype=np.float32)
    ks = kernel * np.float32(SW)
    wh = ks.astype(np.float16)
    wl8 = ((ks - wh.astype(np.float32)) * np.float32(SWL8 / SW)).astype(E4M3)
    wh8 = (wh.astype(np.float32) * np.float32(SWH8 / SW)).astype(E4M3)
    wh_t = np.ascontiguousarray(wh.reshape(KC, 128, E).transpose(1, 0, 2))
    wh8_t = np.ascontiguousarray(wh8.reshape(KC, 128, E).transpose(1, 0, 2))
    wl8_t = np.ascontiguousarray(wl8.reshape(KC, 128, E).transpose(1, 0, 2))
    bb = np.ascontiguousarray(np.broadcast_to(bias, (128, E)))
    in_maps = []
    for core in range(N_CORES):
        xs = x[core * TS:(core + 1) * TS] * np.float32(SX)       # [1024, 7168]
        xsh = xs.astype(np.float16)
        xl8 = ((xs - xsh.astype(np.float32))
               * np.float32(SXL8 / SX)).astype(E4M3)
        xh_t = np.ascontiguousarray(
            xsh.reshape(NT, 128, KC, 128).transpose(0, 3, 2, 1))
        xl8_t = np.ascontiguousarray(
            xl8.reshape(NT, 128, KC, 128).transpose(0, 3, 2, 1))
        in_maps.append({"xh": xh_t, "xl8": xl8_t, "wh": wh_t,
                        "wh8": wh8_t, "wl8": wl8_t, "bb": bb})
    return in_maps


def kernel(x, kernel, bias, _trace=False):
    if "nc" not in _BUILt:
        _BUILt["nc"] = build_nc()
    nc = _BUILt["nc"]
    in_maps = _prep_inputs(x, kernel, bias)
    res = bass_utils.run_bass_kernel_spmd(
        nc, in_maps, core_ids=list(range(N_CORES)), trace=_trace)
    weights = np.empty((T, TOP_K), np.float32)
    idx = np.empty((T, TOP_K), np.int32)
    for core in range(N_CORES):
        weights[core * TS:(core + 1) * TS] = res.results[core]["wout"].reshape(TS, TOP_K)
        idx[core * TS:(core + 1) * TS] = (
            res.results[core]["iout"].reshape(TS, TOP_K).astype(np.int32))
    _BUILt["last_result"] = res
    return weights, idx
